# revision 35
# baseline (speedup 1.0000x reference)
"""Multi-head attention Trainium2 Bass kernel (8 NeuronCores), v4.

Problem: nn_MultiHeadAttention (B=2, S=2048, D=1024, H=16, DK=64).

The reference's raw `.view(B, H, S, DK)` reshape makes head h of batch b a
reinterpretation of the contiguous 128-row block x[b, 128h:128h+128, :], so
each (b, h) is an independent attention problem. 32 pairs over 8 cores ->
4 pairs/core, no collectives. Query/key positions are permuted
(s2' = m*128 + r instead of r*16 + m) identically on q and k (softmax is
permutation-invariant along keys) and un-permuted for free by the ctx
layout.

PE cycle floor (fp16, 2.4 GHz, measured 216 ns per 512-wide matmul):
projections 98304c + scores 131072c + ctx 131072c + out-proj 32768c
~= 169 us — the hard wall. Scalar exp wall ~120 us. The kernel is one
software-pipelined instruction stream tuned to keep the in-order PE FIFO
dense from ~12 us:

 - single-queue priority DMA (parallel queues split HBM bandwidth evenly
   and starve the critical prefix): x & Wk-lo kc-pieces interleaved, Wq-lo,
   Wv-lo, Wk-hi, Wv-hi, Wq-hi, Wo; one SBUF tile per piece so dependency
   tracking unblocks consumers per piece.
 - just-in-time lead-in: only k-c0 + q-c0..3 before the first score;
   k-c1..7, v-chips and q-hi stream into pass 0 at mk hooks.
 - 8 attention passes (pair x query-half); ctx matmuls are emitted with a
   TWO-iteration skew after later scores, so the PE never parks on an exp
   semaphore and sem latency amortizes across the 2-slot sw ring.
 - remaining projections, the deferred normalize/broadcast and per-pair
   output projections are spread as <=8-matmul chips at static mk hooks,
   sized to fill the PE slack under the exp cadence.
 - PSUM (one pending accumulation group per 2KB bank — verified hw
   constraint): sw ring 2x[128,1024] + pcA/pcB accumulator banks + a
   2-bank pj ring for projection/out-proj/broadcast psums.
 - softmax: v65 ones column makes pc row 64 the denominator; den rows
   drain to 32-aligned partitions of one tile; ONE exact [97,512] DVE
   reciprocal per pair (split in half for the last pair so its tail
   shortens); broadcast down 64 partitions via PE outer product
   (ones[1,64].T @ rec16[1,512] -> PSUM, 216 ns); DVE multiplies into the
   ctx layout the out-projection consumes directly. DVE reciprocal is
   ~6.5 cyc/elem, so batching all 2048 denominators of a pair into one
   512-free-dim instruction is 16x cheaper than v1's per-quarter calls;
   reciprocal_approx_fast is broken on this hw (measured ~0.4 rel err).
 - projection drains pair the two same-parity (b,h) pairs into single
   strided [64,2,128] copies; kTz/qT2 are single tensors to enable this.
 - scalar engine runs exp almost exclusively; gpsimd (SBUF-only, 32-
   aligned partition windows) does memsets; DVE does everything else.
 - fp16 output, cast + bias on host: halves the tail DMA.

Measured: 240.7 us (v1 baseline) -> ~218.5 us, rel err 1.2e-3.
fp16 matmul operands (fp32 PSUM); bq/bk/bv are zeros by spec; bo on host.
"""

import sys

sys.path.insert(0, "/opt/trn_rl_repo")

import numpy as np

import concourse.bass as bass  # noqa: E402
import concourse.tile as tile  # noqa: E402
from concourse import bacc, mybir  # noqa: E402
from concourse.bass_utils import run_bass_kernel_spmd  # noqa: E402

F16 = mybir.dt.float16
F32 = mybir.dt.float32

B, S, D, H = 2, 2048, 1024, 16
DK = 64
NCORES = 8
NPAIR = 4
R = 128
NM = 16
S2 = NM * R
KC = D // 128
SCALE = 1.0 / np.sqrt(np.float32(DK))


def _build():
    nc = bacc.Bacc("TRN2", target_bir_lowering=False, debug=False,
                   num_devices=NCORES)

    xTa = nc.dram_tensor("xTa", [128, KC * 512], F16, kind="ExternalInput").ap()
    wqa = nc.dram_tensor("wqa", [128, KC * 1024], F16,
                         kind="ExternalInput").ap()
    wka = nc.dram_tensor("wka", [128, KC * 1024], F16,
                         kind="ExternalInput").ap()
    wva = nc.dram_tensor("wva", [128, KC * 1024], F16,
                         kind="ExternalInput").ap()
    woa = nc.dram_tensor("woa", [128, KC * 1024], F16,
                         kind="ExternalInput").ap()
    out = nc.dram_tensor("out", [NPAIR * R, D], F16, kind="ExternalOutput").ap()

    with tile.TileContext(nc) as tc:
        with tc.tile_pool(name="w", bufs=1) as wpool, \
             tc.tile_pool(name="xp", bufs=1) as xpool, \
             tc.tile_pool(name="qk", bufs=1) as qkpool, \
             tc.tile_pool(name="v6", bufs=1) as vpool, \
             tc.tile_pool(name="pt", bufs=1) as ptpool, \
             tc.tile_pool(name="cu", bufs=1) as cupool, \
             tc.tile_pool(name="cx", bufs=1) as cpool, \
             tc.tile_pool(name="ot", bufs=1) as otpool, \
             tc.tile_pool(name="ps", bufs=1, space="PSUM") as pspool:

            # ---------------- SBUF persistent tiles ----------------
            # one tile per DMA piece: DMA-written tiles get tile-granular
            # read dependencies, so a piece-tile unblocks consumers as soon
            # as ITS transfer lands.
            xTp = [xpool.tile([128, 512 if i < 2 else 1024], F16,
                              name=f"xT{i}", tag=f"x{i}", bufs=1)
                   for i in range(5)]
            wklo = [wpool.tile([128, 512 if i < 2 else 1024], F16,
                               name=f"wklo{i}", tag=f"wkl{i}", bufs=1)
                    for i in range(5)]
            wkhi = wpool.tile([128, 4096], F16, name="wkhi", tag="wkh",
                              bufs=1)
            wqlo = [wpool.tile([128, 2048], F16, name=f"wqlo{i}",
                               tag=f"wql{i}", bufs=1) for i in range(2)]
            wqhi = wpool.tile([128, 4096], F16, name="wqhi", tag="wqh",
                              bufs=1)
            wvh = [wpool.tile([128, 4096], F16, name=f"wv{g}", tag=f"wv{g}",
                              bufs=1) for g in range(2)]
            woh = [wpool.tile([128, 4096], F16, name=f"wo{j}", tag=f"wo{j}",
                              bufs=1) for j in range(2)]

            def _pc_off(kc):
                """(piece index, col offset) — kc 0/1 are half pieces."""
                return (kc, 0) if kc < 2 else (kc // 2 + 1, (kc % 2) * 512)

            def xop(kc):
                i, o = _pc_off(kc)
                return xTp[i][:, o:o + 512]

            def xop_pr(kc, pr):
                i, o = _pc_off(kc)
                return xTp[i][:, o + pr * 128:o + (pr + 1) * 128]

            def wk_op(c, kc):
                if c < 4:
                    i, o = _pc_off(kc)
                    return wklo[i][:, o + c * 128:o + (c + 1) * 128]
                o = kc * 512 + (c - 4) * 128
                return wkhi[:, o:o + 128]

            def wq_op(c, kc):
                if c < 4:
                    o = (kc % 4) * 512 + c * 128
                    return wqlo[kc // 4][:, o:o + 128]
                o = kc * 512 + (c - 4) * 128
                return wqhi[:, o:o + 128]

            qT2a = qkpool.tile([128, 2 * S2], F16, name="qT2a", tag="q",
                               bufs=1)
            kTza = qkpool.tile([128, NPAIR * S2], F16, name="kTza", tag="kz",
                               bufs=1)
            v65 = [vpool.tile([128, NM * 65], F16, name=f"v65{p}",
                              tag=f"v{p}", bufs=1) for p in range(NPAIR)]
            ctx = [cpool.tile([128, D], F16, name=f"ctx{p}", tag=f"c{p}",
                              bufs=1) for p in range(NPAIR)]
            ones64 = cupool.tile([97, 64], F16, name="ones64", tag="o64",
                                 bufs=1)
            nc.vector.memset(ones64[:], 1.0)

            # ---------------- input DMA ----------------
            # weights are host-packed half-major: cols [h*4096 + kc*512+f],
            # h = output-feature half. All pieces contiguous (fast issue),
            # priority: x, Wk lo | Wq lo, Wv lo, Wk hi, Wv hi | Wq hi, Wo.
            # one queue, strict priority order (parallel queues split the
            # HBM bandwidth evenly and starve the critical prefix)
            bounds = [(0, 512), (512, 1024), (1024, 2048), (2048, 3072),
                      (3072, 4096)]
            for i, (a, b) in enumerate(bounds):
                nc.sync.dma_start(xTp[i][:], xTa[:, a:b])
                nc.sync.dma_start(wklo[i][:], wka[:, a:b])
            for i in range(2):
                nc.sync.dma_start(wqlo[i][:],
                                  wqa[:, i * 2048:(i + 1) * 2048])
            nc.sync.dma_start(wvh[0][:], wva[:, 0:4096])
            nc.sync.dma_start(wkhi[:], wka[:, 4096:8192])
            nc.sync.dma_start(wvh[1][:], wva[:, 4096:8192])
            nc.sync.dma_start(wqhi[:], wqa[:, 4096:8192])
            nc.sync.dma_start(woh[0][:], woa[:, 0:4096])
            nc.sync.dma_start(woh[1][:], woa[:, 4096:8192])

            # kTz zero halves (the other pair's q rows multiply zeros) and
            # v65 ones columns (free softmax denominators).
            for p in range(NPAIR):
                other = (1 - (p % 2)) * 64
                nc.gpsimd.memset(
                    kTza[other:other + 64, p * S2:(p + 1) * S2], 0.0)
            for p in range(NPAIR):
                ones_cols = v65[p][:].rearrange("p (m c) -> p m c",
                                                m=NM)[:, :, 64:65]
                nc.gpsimd.memset(ones_cols, 1.0)

            # ---------------- projection helpers ----------------
            chip_ps = {}

            def qk_chip(w_op, c, dst_tiles, tag, kc0, kc1, scalar_ok=False):
                """Emit kc0..kc1 of one qk projection chunk; drains on the
                last chip."""
                key = (w_op, c)
                if kc0 == 0:
                    chip_ps[key] = pspool.tile(
                        [128, 512], F32, name=f"pp{c}", tag=tag,
                        bufs=2 if tag in ("sc", "pj") else 1)
                ps = chip_ps[key]
                for kc in range(kc0, kc1):
                    nc.tensor.matmul(
                        ps[:], w_op(c, kc), xop(kc),
                        start=(kc == 0), stop=(kc == KC - 1))
                if kc1 == KC:
                    qk_drain(ps, c, dst_tiles, scalar_ok)

            def qk_drain(ps, c, dst_t, scalar_ok):
                """Paired drains: pairs (par, par+2) share the partition
                window, so each (mp, par) is ONE strided [64,2,128] copy.
                dst_t: (tile, pair-stride in columns)."""
                tile_t, mode = dst_t
                for mp in range(2):
                    m = 2 * c + mp
                    for par in range(2):
                        srcv = ps[mp * 64:mp * 64 + 64, :].rearrange(
                            "p (a g r) -> p a g r", a=2, g=2)[:, :, par, :]
                        # pair pr = a*2 + par; k dest: col pr*S2 + m*128;
                        # q dest: col (pr//2)*S2 + m*128 = a*S2 + m*128
                        base = par * S2 if mode == "k" else 0
                        dstv = tile_t[par * 64:par * 64 + 64, :].rearrange(
                            "p (a rest) -> p a rest", a=2)[
                            :, :, base + m * 128:base + m * 128 + 128]
                        if mp == 0 or not scalar_ok:
                            nc.vector.tensor_copy(dstv, srcv)
                        else:
                            nc.scalar.copy(dstv, srcv)

            def v_chip(pr, g, tag, kc0, kc1):
                key = ("v", pr, g)
                if kc0 == 0:
                    chip_ps[key] = pspool.tile(
                        [128, 512], F32, name=f"pv{pr}{g}", tag=tag,
                        bufs=2 if tag in ("sc", "pj") else 1)
                psv = chip_ps[key]
                for kc in range(kc0, kc1):
                    nc.tensor.matmul(
                        psv[:], xop_pr(kc, pr),
                        wvh[g][:, kc * 512:(kc + 1) * 512],
                        start=(kc == 0), stop=(kc == KC - 1))
                if kc1 == KC:
                    dst = v65[pr][:].rearrange(
                        "p (m c) -> p m c", m=NM)[:, g * 8:(g + 1) * 8, 0:64]
                    nc.vector.tensor_copy(
                        dst, psv[:].rearrange("p (m c) -> p m c", m=8))

            def op_chip(pr, jb, c0, c1, tag="pj"):
                key = ("o", pr, jb)
                if c0 == 0:
                    chip_ps[key] = pspool.tile([128, 512], F32, name="po",
                                               tag=tag, bufs=2)
                po = chip_ps[key]
                for c in range(c0, c1):
                    nc.tensor.matmul(
                        po[:],
                        ctx[pr][:, c * 128:(c + 1) * 128],
                        woh[jb][:, c * 512:(c + 1) * 512],
                        start=(c == 0), stop=(c == KC - 1))
                if c1 == KC:
                    ot = otpool.tile([128, 512], F16, name="ot", tag="ot",
                                     bufs=4)
                    nc.vector.tensor_copy(ot[:], po[:])
                    nc.sync.dma_start(
                        out[pr * 128:(pr + 1) * 128,
                            jb * 512:(jb + 1) * 512], ot[:])

            # ---------------- deferred pair-tail work ----------------
            cu = {}
            dens = {}
            state = {}

            def norm_recip(pr):
                """Batched reciprocal of pair pr's denominators + fp16
                cast + qs=3 relay (PE operand base must be 0/32/64)."""
                rec = cupool.tile([97, 512], F32, name="rec", tag="rec",
                                  bufs=2)
                nc.vector.reciprocal(rec[:], dens[pr][:])
                rec16 = cupool.tile([97, 512], F16, name="rec16", tag="r16",
                                    bufs=2)
                nc.vector.tensor_copy(rec16[:], rec[:])
                rec16b = cupool.tile([1, 512], F16, name="rec16b",
                                     tag="r16b", bufs=2)
                nc.vector.tensor_copy(rec16b[:], rec16[96:97, :])
                state[pr] = (rec16, rec16b)

            def norm_recip_part(pr, part):
                """Half-pair reciprocal (rows 0/32 or 64/96) so the last
                pair's first two quarters normalize during its final pass."""
                if part == 0:
                    rec = cupool.tile([97, 512], F32, name="rec", tag="rec",
                                      bufs=2)
                    rec16 = cupool.tile([97, 512], F16, name="rec16",
                                        tag="r16", bufs=2)
                    rec16b = cupool.tile([1, 512], F16, name="rec16b",
                                         tag="r16b", bufs=2)
                    state[pr] = (rec16, rec16b)
                    state[(pr, "rec")] = rec
                    nc.vector.reciprocal(rec[0:33, :], dens[pr][0:33, :])
                    nc.vector.tensor_copy(rec16[0:33, :], rec[0:33, :])
                else:
                    rec = state[(pr, "rec")]
                    rec16, rec16b = state[pr]
                    nc.vector.reciprocal(rec[64:97, :], dens[pr][64:97, :])
                    nc.vector.tensor_copy(rec16[64:97, :], rec[64:97, :])
                    nc.vector.tensor_copy(rec16b[:], rec16[96:97, :])

            def norm_qs(pr, qs, tag="pj"):
                """PE outer-product broadcast of 1/den + DVE multiply into
                the ctx layout."""
                rec16, rec16b = state[pr]
                pbs = pspool.tile([64, 512], F32, name="pbs", tag=tag,
                                  bufs=2)
                lo = qs * 32 if qs < 3 else 0
                rsrc = rec16[lo:lo + 1, :] if qs < 3 else rec16b[:]
                nc.tensor.matmul(pbs[:], ones64[lo:lo + 1, :], rsrc,
                                 start=True, stop=True)
                for p2 in range(2):
                    src = cu[(pr, qs)][:].rearrange(
                        "p (a q c) -> p a q c", a=2, q=2)[:, :, p2, :]
                    bb = pbs[:].rearrange(
                        "p (a q c) -> p a q c", a=2, q=2)[:, :, p2, :]
                    dst = ctx[pr][p2 * 64:(p2 + 1) * 64,
                                  qs * 256:(qs + 1) * 256].rearrange(
                        "p (a c) -> p a c", a=2)
                    nc.vector.tensor_mul(dst, src, bb)

            def outproj(pr, jb):
                po = pspool.tile([128, 512], F32, name="po", tag="pj",
                                 bufs=2)
                for c in range(KC):
                    nc.tensor.matmul(
                        po[:],
                        ctx[pr][:, c * 128:(c + 1) * 128],
                        wo[:, c * 1024 + jb * 512:c * 1024 + (jb + 1) * 512],
                        start=(c == 0), stop=(c == KC - 1))
                ot = otpool.tile([128, 512], F16, name="ot", tag="ot",
                                 bufs=4)
                nc.vector.tensor_copy(ot[:], po[:])
                nc.sync.dma_start(
                    out[pr * 128:(pr + 1) * 128,
                        jb * 512:(jb + 1) * 512], ot[:])

            # ---------------- phase 1 lead-in ----------------
            # k c0..3 in kc-halves (streams behind the split wkA DMA,
            # 4 open psums), then q c0..3, then v(pair0) lo half.
            KDST = (kTza, "k")    # pair p at columns p*S2
            QDST = (qT2a, "q")    # t = p//2 at columns t*S2
            qk_chip(wk_op, 0, KDST, "sc", 0, 4, scalar_ok=True)
            qk_chip(wk_op, 0, KDST, "sc", 4, 8, scalar_ok=True)
            for c, tg in zip(range(4), ("pcA", "pcB", "sc", "pj")):
                qk_chip(wq_op, c, QDST, tg, 0, 4, scalar_ok=True)
            for c, tg in zip(range(4), ("pcA", "pcB", "sc", "pj")):
                qk_chip(wq_op, c, QDST, tg, 4, 8, scalar_ok=True)

            # ------------- static fill schedule (pass, mk) -> thunks -----
            # Every chip is <= 8 matmuls; placed so its inputs (DMA pieces,
            # the DVE reciprocal chain, normalized ctx) are ready before
            # the PE FIFO reaches it.
            SCHED = {}

            def at(pi, mk, fn):
                SCHED.setdefault((pi, mk), []).append(fn)

            # pass 0 (pr0-h0): k c1..3 and v0-lo just-in-time, then the
            # hi halves as their DMA pieces land.
            for i, c in enumerate((1, 2, 3)):
                at(0, i, lambda c=c: qk_chip(wk_op, c, KDST, "pj", 0, 8))
            at(0, 0, lambda: v_chip(0, 0, "pj", 0, 4))
            at(0, 1, lambda: v_chip(0, 0, "pj", 4, 8))
            for i, c in enumerate(range(4, 8)):
                at(0, 5 + i, lambda c=c: qk_chip(wk_op, c, KDST, "pj", 0, 8))
            at(0, 8, lambda: v_chip(0, 1, "pj", 0, 4))
            at(0, 9, lambda: v_chip(0, 1, "pj", 4, 8))
            for i, c in enumerate(range(4, 8)):
                at(0, 10 + i,
                   lambda c=c: qk_chip(wq_op, c, QDST, "pj", 0, 8))
            # v projections for later pairs, two chips each
            for p in range(1, NPAIR):
                pi0 = (p - 1) * 2 + 1          # previous pair's h1 pass
                at(pi0, 1, lambda p=p: v_chip(p, 0, "pj", 0, 4))
                at(pi0, 2, lambda p=p: v_chip(p, 0, "pj", 4, 8))
                at(pi0, 8, lambda p=p: v_chip(p, 1, "pj", 0, 4))
                at(pi0, 9, lambda p=p: v_chip(p, 1, "pj", 4, 8))
            # previous pair's normalize + out-projection (recip chain is
            # kicked off at this pass's mk0; ready by ~mk5)
            for pr in range(NPAIR - 1):
                pi = (pr + 1) * 2
                for qs in range(4):
                    at(pi, 8 + qs, lambda pr=pr, qs=qs: norm_qs(pr, qs))
                at(pi, 12, lambda pr=pr: op_chip(pr, 0, 0, 4))
                at(pi, 13, lambda pr=pr: op_chip(pr, 0, 4, 8))
                if pr < 2:
                    at(pi, 14, lambda pr=pr: op_chip(pr, 1, 0, 4))
                    at(pi, 15, lambda pr=pr: op_chip(pr, 1, 4, 8))
            # last pair: quarters 0/1 normalize inside its final pass and
            # the out-projection's low c-chunks (ctx cols 0:512) follow
            at(7, 7, lambda: norm_qs(3, 0))
            at(7, 8, lambda: norm_qs(3, 1))
            at(7, 11, lambda: op_chip(3, 0, 0, 4))
            at(7, 13, lambda: op_chip(3, 1, 0, 4))

            # ------------- attention: 8 skewed passes --------------------
            def drain_pass(pc, pr, h2):
                """Drains of a finished pass: den rows (scalar), ctx rows
                (DVE, ahead of the reciprocal so the next pass's ctx isn't
                queued behind it), then the reciprocal chain."""
                if h2 == 0:
                    dens[pr] = cupool.tile([97, 512], F32, name="den",
                                           tag="den", bufs=2)
                for qh in range(2):
                    qs = h2 * 2 + qh
                    nc.vector.tensor_copy(dens[pr][qs * 32:qs * 32 + 1, :],
                                          pc[qh][64:65, :])
                    c_t = cupool.tile([64, 512], F32, name=f"cu{pr}{qs}",
                                      tag=f"cu{qs}", bufs=2)
                    nc.vector.tensor_copy(c_t[:], pc[qh][0:64, :])
                    cu[(pr, qs)] = c_t
                if pr == NPAIR - 1:
                    norm_recip_part(pr, h2)   # split halves for the tail
                elif h2 == 1:
                    norm_recip(pr)

            # two-iteration ctx skew: emit ctx(i-2) after scores(i), so
            # the in-order PE FIFO never parks on an exp semaphore and the
            # sem latency amortizes across the 2-slot sw ring.
            pend = []          # [(thunk, pass_info_if_last | None), ...]
            for pi in range(2 * NPAIR):
                pr, h2 = pi // 2, pi % 2
                t = pr // 2
                pc = [pspool.tile([65, 512], F32, name=f"pc{qh}",
                                  tag=("pcA", "pcB")[qh], bufs=1)
                      for qh in range(2)]
                for mk in range(NM):
                    sw = pspool.tile([128, 1024], F32, name="sw",
                                     tag="sc", bufs=2)
                    for qh in range(2):
                        nc.tensor.matmul(
                            sw[:, qh * 512:(qh + 1) * 512],
                            kTza[:, pr * S2 + mk * 128:
                                 pr * S2 + (mk + 1) * 128],
                            qT2a[:, (t * S2 + h2 * 1024 + qh * 512):
                                 (t * S2 + h2 * 1024 + (qh + 1) * 512)],
                            start=True, stop=True)
                    pT = ptpool.tile([128, 1024], F16, name="pT",
                                     tag="pt", bufs=6)
                    nc.scalar.activation(
                        pT[:], sw[:], mybir.ActivationFunctionType.Exp,
                        scale=float(SCALE))
                    if len(pend) == 2:
                        fn, fin = pend.pop(0)
                        fn()
                        if fin is not None:
                            drain_pass(*fin)

                    def make_ctx(pc=pc, pr=pr, mk=mk, pT=pT):
                        for qh in range(2):
                            nc.tensor.matmul(
                                pc[qh][:],
                                v65[pr][:, mk * 65:(mk + 1) * 65],
                                pT[:, qh * 512:(qh + 1) * 512],
                                start=(mk == 0), stop=(mk == NM - 1))
                    pend.append((make_ctx,
                                 (pc, pr, h2) if mk == NM - 1 else None))
                    for fn in SCHED.get((pi, mk), []):
                        fn()

            # flush remaining skewed ctx + drains (kicks recip part 1)
            for fn, fin in pend:
                fn()
                if fin is not None:
                    drain_pass(*fin)
            op_chip(2, 1, 0, 8, tag="sc")
            norm_qs(3, 2, tag="sc")
            norm_qs(3, 3, tag="sc")
            op_chip(3, 0, 4, 8)
            op_chip(3, 1, 4, 8)

    nc.compile()
    return nc


_CACHE = {}


def _get_nc():
    if "nc" not in _CACHE:
        _CACHE["nc"] = _build()
    return _CACHE["nc"]


def _kc_block(a, cols):
    """[1024, cols] -> [128, 8*cols] with kc blocks along columns."""
    return np.ascontiguousarray(
        a.reshape(KC, 128, cols).transpose(1, 0, 2).reshape(128, KC * cols))


def _half_block(a):
    """[1024, 1024] W.T -> [128, 8192] half-major: col h*4096 + kc*512 + f
    holds W.T[kc*128+p, h*512+f]."""
    return np.ascontiguousarray(
        a.reshape(KC, 128, 2, 512).transpose(1, 2, 0, 3).reshape(128, 8192))


def _prep_inputs(x, Wq, Wk, Wv, Wo):
    x = np.asarray(x, dtype=np.float32)
    wqa = _half_block(np.ascontiguousarray(Wq.T, dtype=np.float16))
    wka = _half_block(np.ascontiguousarray(Wk.T, dtype=np.float16))
    wva = _half_block(np.ascontiguousarray(Wv.T, dtype=np.float16))
    woa = _half_block(np.ascontiguousarray(Wo.T, dtype=np.float16))

    in_maps = []
    for core in range(NCORES):
        b, hg = core // 4, core % 4
        rows = x[b, hg * 512:(hg + 1) * 512, :]
        xTa = _kc_block(np.ascontiguousarray(rows.T.astype(np.float16)), 512)
        in_maps.append({
            "xTa": xTa, "wqa": wqa, "wka": wka, "wva": wva, "woa": woa,
        })
    return in_maps


def _run(in_maps, trace=False):
    nc = _get_nc()
    return run_bass_kernel_spmd(nc, in_maps, core_ids=list(range(NCORES)),
                                trace=trace)


def kernel(x, Wq, bq, Wk, bk, Wv, bv, Wo, bo, _trace=False):
    x = np.asarray(x, dtype=np.float32)
    in_maps = _prep_inputs(x, np.asarray(Wq), np.asarray(Wk),
                           np.asarray(Wv), np.asarray(Wo))
    res = _run(in_maps, trace=_trace)
    out = np.empty((B, S, D), dtype=np.float32)
    for core in range(NCORES):
        b, hg = core // 4, core % 4
        out[b, hg * 512:(hg + 1) * 512, :] = res.results[core]["out"]
    out += np.asarray(bo, dtype=np.float32)[None, None, :]
    kernel.last_result = res
    return out


# revision 37
# speedup vs baseline: 1.0045x; 1.0045x over previous
"""Multi-head attention Trainium2 Bass kernel (8 NeuronCores), v4.

Problem: nn_MultiHeadAttention (B=2, S=2048, D=1024, H=16, DK=64).

The reference's raw `.view(B, H, S, DK)` reshape makes head h of batch b a
reinterpretation of the contiguous 128-row block x[b, 128h:128h+128, :], so
each (b, h) is an independent attention problem. 32 pairs over 8 cores ->
4 pairs/core, no collectives. Query/key positions are permuted
(s2' = m*128 + r instead of r*16 + m) identically on q and k (softmax is
permutation-invariant along keys) and un-permuted for free by the ctx
layout.

PE cycle floor (fp16, 2.4 GHz, measured 216 ns per 512-wide matmul):
projections 98304c + scores 131072c + ctx 131072c + out-proj 32768c
~= 169 us — the hard wall. Scalar exp wall ~120 us. The kernel is one
software-pipelined instruction stream tuned to keep the in-order PE FIFO
dense from ~12 us:

 - single-queue priority DMA (parallel queues split HBM bandwidth evenly
   and starve the critical prefix): x & Wk-lo kc-pieces interleaved, Wq-lo,
   Wv-lo, Wk-hi, Wv-hi, Wq-hi, Wo; one SBUF tile per piece so dependency
   tracking unblocks consumers per piece.
 - just-in-time lead-in: only k-c0 + q-c0..3 before the first score;
   k-c1..7, v-chips and q-hi stream into pass 0 at mk hooks.
 - 8 attention passes (pair x query-half); ctx matmuls are emitted with a
   TWO-iteration skew after later scores, so the PE never parks on an exp
   semaphore and sem latency amortizes across the 2-slot sw ring.
 - remaining projections, the deferred normalize/broadcast and per-pair
   output projections are spread as <=8-matmul chips at static mk hooks,
   sized to fill the PE slack under the exp cadence.
 - PSUM (one pending accumulation group per 2KB bank — verified hw
   constraint): sw ring 2x[128,1024] + pcA/pcB accumulator banks + a
   2-bank pj ring for projection/out-proj/broadcast psums.
 - softmax: v65 ones column makes pc row 64 the denominator; den rows
   drain to 32-aligned partitions of one tile; ONE exact [97,512] DVE
   reciprocal per pair (split in half for the last pair so its tail
   shortens); broadcast down 64 partitions via PE outer product
   (ones[1,64].T @ rec16[1,512] -> PSUM, 216 ns); DVE multiplies into the
   ctx layout the out-projection consumes directly. DVE reciprocal is
   ~6.5 cyc/elem, so batching all 2048 denominators of a pair into one
   512-free-dim instruction is 16x cheaper than v1's per-quarter calls;
   reciprocal_approx_fast is broken on this hw (measured ~0.4 rel err).
 - projection drains pair the two same-parity (b,h) pairs into single
   strided [64,2,128] copies; kTz/qT2 are single tensors to enable this.
 - scalar engine runs exp almost exclusively; gpsimd (SBUF-only, 32-
   aligned partition windows) does memsets; DVE does everything else.
 - fp16 output, cast + bias on host: halves the tail DMA.

Measured: 240.7 us (v1 baseline) -> ~218.5 us, rel err 1.2e-3.
fp16 matmul operands (fp32 PSUM); bq/bk/bv are zeros by spec; bo on host.
"""

import sys

sys.path.insert(0, "/opt/trn_rl_repo")

import numpy as np

import concourse.bass as bass  # noqa: E402
import concourse.tile as tile  # noqa: E402
from concourse import bacc, mybir  # noqa: E402
from concourse.bass_utils import run_bass_kernel_spmd  # noqa: E402

F16 = mybir.dt.float16
F32 = mybir.dt.float32

B, S, D, H = 2, 2048, 1024, 16
DK = 64
NCORES = 8
NPAIR = 4
R = 128
NM = 16
S2 = NM * R
KC = D // 128
SCALE = 1.0 / np.sqrt(np.float32(DK))


def _build():
    nc = bacc.Bacc("TRN2", target_bir_lowering=False, debug=False,
                   num_devices=NCORES)

    xTa = nc.dram_tensor("xTa", [128, KC * 512], F16, kind="ExternalInput").ap()
    wqa = nc.dram_tensor("wqa", [128, KC * 1024], F16,
                         kind="ExternalInput").ap()
    wka = nc.dram_tensor("wka", [128, KC * 1024], F16,
                         kind="ExternalInput").ap()
    wva = nc.dram_tensor("wva", [128, KC * 1024], F16,
                         kind="ExternalInput").ap()
    woa = nc.dram_tensor("woa", [128, KC * 1024], F16,
                         kind="ExternalInput").ap()
    out = nc.dram_tensor("out", [NPAIR * R, D], F16, kind="ExternalOutput").ap()

    with tile.TileContext(nc) as tc:
        with tc.tile_pool(name="w", bufs=1) as wpool, \
             tc.tile_pool(name="xp", bufs=1) as xpool, \
             tc.tile_pool(name="qk", bufs=1) as qkpool, \
             tc.tile_pool(name="v6", bufs=1) as vpool, \
             tc.tile_pool(name="pt", bufs=1) as ptpool, \
             tc.tile_pool(name="cu", bufs=1) as cupool, \
             tc.tile_pool(name="cx", bufs=1) as cpool, \
             tc.tile_pool(name="ot", bufs=1) as otpool, \
             tc.tile_pool(name="ps", bufs=1, space="PSUM") as pspool:

            # ---------------- SBUF persistent tiles ----------------
            # one tile per DMA piece: DMA-written tiles get tile-granular
            # read dependencies, so a piece-tile unblocks consumers as soon
            # as ITS transfer lands.
            xTp = [xpool.tile([128, 1024], F16, name=f"xT{i}", tag=f"x{i}",
                              bufs=1) for i in range(4)]
            wklo = [wpool.tile([128, 1024], F16, name=f"wklo{i}",
                               tag=f"wkl{i}", bufs=1) for i in range(4)]
            wkhi = wpool.tile([128, 4096], F16, name="wkhi", tag="wkh",
                              bufs=1)
            wqlo = [wpool.tile([128, 2048], F16, name=f"wqlo{i}",
                               tag=f"wql{i}", bufs=1) for i in range(2)]
            wqhi = wpool.tile([128, 4096], F16, name="wqhi", tag="wqh",
                              bufs=1)
            wvh = [wpool.tile([128, 4096], F16, name=f"wv{g}", tag=f"wv{g}",
                              bufs=1) for g in range(2)]
            woh = [wpool.tile([128, 4096], F16, name=f"wo{j}", tag=f"wo{j}",
                              bufs=1) for j in range(2)]

            def xop(kc):
                return xTp[kc // 2][:, (kc % 2) * 512:(kc % 2 + 1) * 512]

            def xop_pr(kc, pr):
                o = (kc % 2) * 512 + pr * 128
                return xTp[kc // 2][:, o:o + 128]

            def wk_op(c, kc):
                if c < 4:
                    o = (kc % 2) * 512 + c * 128
                    return wklo[kc // 2][:, o:o + 128]
                o = kc * 512 + (c - 4) * 128
                return wkhi[:, o:o + 128]

            def wq_op(c, kc):
                if c < 4:
                    o = (kc % 4) * 512 + c * 128
                    return wqlo[kc // 4][:, o:o + 128]
                o = kc * 512 + (c - 4) * 128
                return wqhi[:, o:o + 128]

            qT2a = qkpool.tile([128, 2 * S2], F16, name="qT2a", tag="q",
                               bufs=1)
            kTza = qkpool.tile([128, NPAIR * S2], F16, name="kTza", tag="kz",
                               bufs=1)
            v65 = [vpool.tile([128, NM * 65], F16, name=f"v65{p}",
                              tag=f"v{p}", bufs=1) for p in range(NPAIR)]
            ctx = [cpool.tile([128, D], F16, name=f"ctx{p}", tag=f"c{p}",
                              bufs=1) for p in range(NPAIR)]
            ones64 = cupool.tile([97, 64], F16, name="ones64", tag="o64",
                                 bufs=1)
            nc.vector.memset(ones64[:], 1.0)

            # ---------------- input DMA ----------------
            # weights are host-packed half-major: cols [h*4096 + kc*512+f],
            # h = output-feature half. All pieces contiguous (fast issue),
            # priority: x, Wk lo | Wq lo, Wv lo, Wk hi, Wv hi | Wq hi, Wo.
            # one queue, strict priority order (parallel queues split the
            # HBM bandwidth evenly and starve the critical prefix)
            for i in range(4):
                nc.sync.dma_start(xTp[i][:], xTa[:, i * 1024:(i + 1) * 1024])
                nc.sync.dma_start(wklo[i][:],
                                  wka[:, i * 1024:(i + 1) * 1024])
            for i in range(2):
                nc.sync.dma_start(wqlo[i][:],
                                  wqa[:, i * 2048:(i + 1) * 2048])
            nc.sync.dma_start(wvh[0][:], wva[:, 0:4096])
            nc.sync.dma_start(wkhi[:], wka[:, 4096:8192])
            nc.sync.dma_start(wvh[1][:], wva[:, 4096:8192])
            nc.sync.dma_start(wqhi[:], wqa[:, 4096:8192])


            # kTz zero halves (the other pair's q rows multiply zeros) and
            # v65 ones columns (free softmax denominators).
            for p in range(NPAIR):
                other = (1 - (p % 2)) * 64
                nc.gpsimd.memset(
                    kTza[other:other + 64, p * S2:(p + 1) * S2], 0.0)
            for p in range(NPAIR):
                ones_cols = v65[p][:].rearrange("p (m c) -> p m c",
                                                m=NM)[:, :, 64:65]
                nc.gpsimd.memset(ones_cols, 1.0)
            dmy = cupool.tile([1, 512], F16, name="dmy", tag="dmy", bufs=1)
            nc.gpsimd.tensor_copy(dmy[:], qT2a[0:1, 0:512])
            nc.gpsimd.dma_start(woh[0][:], woa[:, 0:4096])
            nc.gpsimd.dma_start(woh[1][:], woa[:, 4096:8192])

            # ---------------- projection helpers ----------------
            chip_ps = {}

            def qk_chip(w_op, c, dst_tiles, tag, kc0, kc1, scalar_ok=False):
                """Emit kc0..kc1 of one qk projection chunk; drains on the
                last chip."""
                key = (w_op, c)
                if kc0 == 0:
                    chip_ps[key] = pspool.tile(
                        [128, 512], F32, name=f"pp{c}", tag=tag,
                        bufs=2 if tag in ("sc", "pj") else 1)
                ps = chip_ps[key]
                for kc in range(kc0, kc1):
                    nc.tensor.matmul(
                        ps[:], w_op(c, kc), xop(kc),
                        start=(kc == 0), stop=(kc == KC - 1))
                if kc1 == KC:
                    qk_drain(ps, c, dst_tiles, scalar_ok)

            def qk_drain(ps, c, dst_t, scalar_ok):
                """Paired drains: pairs (par, par+2) share the partition
                window, so each (mp, par) is ONE strided [64,2,128] copy.
                dst_t: (tile, pair-stride in columns)."""
                tile_t, mode = dst_t
                for mp in range(2):
                    m = 2 * c + mp
                    for par in range(2):
                        srcv = ps[mp * 64:mp * 64 + 64, :].rearrange(
                            "p (a g r) -> p a g r", a=2, g=2)[:, :, par, :]
                        # pair pr = a*2 + par; k dest: col pr*S2 + m*128;
                        # q dest: col (pr//2)*S2 + m*128 = a*S2 + m*128
                        base = par * S2 if mode == "k" else 0
                        dstv = tile_t[par * 64:par * 64 + 64, :].rearrange(
                            "p (a rest) -> p a rest", a=2)[
                            :, :, base + m * 128:base + m * 128 + 128]
                        if mp == 0 or not scalar_ok:
                            nc.vector.tensor_copy(dstv, srcv)
                        else:
                            nc.scalar.copy(dstv, srcv)

            def v_chip(pr, g, tag, kc0, kc1):
                key = ("v", pr, g)
                if kc0 == 0:
                    chip_ps[key] = pspool.tile(
                        [128, 512], F32, name=f"pv{pr}{g}", tag=tag,
                        bufs=2 if tag in ("sc", "pj") else 1)
                psv = chip_ps[key]
                for kc in range(kc0, kc1):
                    nc.tensor.matmul(
                        psv[:], xop_pr(kc, pr),
                        wvh[g][:, kc * 512:(kc + 1) * 512],
                        start=(kc == 0), stop=(kc == KC - 1))
                if kc1 == KC:
                    dst = v65[pr][:].rearrange(
                        "p (m c) -> p m c", m=NM)[:, g * 8:(g + 1) * 8, 0:64]
                    nc.vector.tensor_copy(
                        dst, psv[:].rearrange("p (m c) -> p m c", m=8))

            def op_chip(pr, jb, c0, c1):
                key = ("o", pr, jb)
                if c0 == 0:
                    chip_ps[key] = pspool.tile([128, 512], F32, name="po",
                                               tag="pj", bufs=2)
                po = chip_ps[key]
                for c in range(c0, c1):
                    nc.tensor.matmul(
                        po[:],
                        ctx[pr][:, c * 128:(c + 1) * 128],
                        woh[jb][:, c * 512:(c + 1) * 512],
                        start=(c == 0), stop=(c == KC - 1))
                if c1 == KC:
                    ot = otpool.tile([128, 512], F16, name="ot", tag="ot",
                                     bufs=4)
                    nc.vector.tensor_copy(ot[:], po[:])
                    nc.sync.dma_start(
                        out[pr * 128:(pr + 1) * 128,
                            jb * 512:(jb + 1) * 512], ot[:])

            # ---------------- deferred pair-tail work ----------------
            cu = {}
            dens = {}
            state = {}

            def norm_recip(pr):
                """Batched reciprocal of pair pr's denominators + fp16
                cast + qs=3 relay (PE operand base must be 0/32/64)."""
                rec = cupool.tile([97, 512], F32, name="rec", tag="rec",
                                  bufs=2)
                nc.vector.reciprocal(rec[:], dens[pr][:])
                rec16 = cupool.tile([97, 512], F16, name="rec16", tag="r16",
                                    bufs=2)
                nc.vector.tensor_copy(rec16[:], rec[:])
                rec16b = cupool.tile([1, 512], F16, name="rec16b",
                                     tag="r16b", bufs=2)
                nc.vector.tensor_copy(rec16b[:], rec16[96:97, :])
                state[pr] = (rec16, rec16b)

            def norm_recip_part(pr, part):
                """Half-pair reciprocal (rows 0/32 or 64/96) so the last
                pair's first two quarters normalize during its final pass."""
                if part == 0:
                    rec = cupool.tile([97, 512], F32, name="rec", tag="rec",
                                      bufs=2)
                    rec16 = cupool.tile([97, 512], F16, name="rec16",
                                        tag="r16", bufs=2)
                    rec16b = cupool.tile([1, 512], F16, name="rec16b",
                                         tag="r16b", bufs=2)
                    state[pr] = (rec16, rec16b)
                    state[(pr, "rec")] = rec
                    nc.vector.reciprocal(rec[0:33, :], dens[pr][0:33, :])
                    nc.vector.tensor_copy(rec16[0:33, :], rec[0:33, :])
                else:
                    rec = state[(pr, "rec")]
                    rec16, rec16b = state[pr]
                    nc.vector.reciprocal(rec[64:97, :], dens[pr][64:97, :])
                    nc.vector.tensor_copy(rec16[64:97, :], rec[64:97, :])
                    nc.vector.tensor_copy(rec16b[:], rec16[96:97, :])

            def norm_qs(pr, qs, tag="pj"):
                """PE outer-product broadcast of 1/den + DVE multiply into
                the ctx layout."""
                rec16, rec16b = state[pr]
                pbs = pspool.tile([64, 512], F32, name="pbs", tag=tag,
                                  bufs=2)
                lo = qs * 32 if qs < 3 else 0
                rsrc = rec16[lo:lo + 1, :] if qs < 3 else rec16b[:]
                nc.tensor.matmul(pbs[:], ones64[lo:lo + 1, :], rsrc,
                                 start=True, stop=True)
                for p2 in range(2):
                    src = cu[(pr, qs)][:].rearrange(
                        "p (a q c) -> p a q c", a=2, q=2)[:, :, p2, :]
                    bb = pbs[:].rearrange(
                        "p (a q c) -> p a q c", a=2, q=2)[:, :, p2, :]
                    dst = ctx[pr][p2 * 64:(p2 + 1) * 64,
                                  qs * 256:(qs + 1) * 256].rearrange(
                        "p (a c) -> p a c", a=2)
                    nc.vector.tensor_mul(dst, src, bb)

            def outproj(pr, jb):
                po = pspool.tile([128, 512], F32, name="po", tag="pj",
                                 bufs=2)
                for c in range(KC):
                    nc.tensor.matmul(
                        po[:],
                        ctx[pr][:, c * 128:(c + 1) * 128],
                        wo[:, c * 1024 + jb * 512:c * 1024 + (jb + 1) * 512],
                        start=(c == 0), stop=(c == KC - 1))
                ot = otpool.tile([128, 512], F16, name="ot", tag="ot",
                                 bufs=4)
                nc.vector.tensor_copy(ot[:], po[:])
                nc.sync.dma_start(
                    out[pr * 128:(pr + 1) * 128,
                        jb * 512:(jb + 1) * 512], ot[:])

            # ---------------- phase 1 lead-in ----------------
            # k c0..3 in kc-halves (streams behind the split wkA DMA,
            # 4 open psums), then q c0..3, then v(pair0) lo half.
            KDST = (kTza, "k")    # pair p at columns p*S2
            QDST = (qT2a, "q")    # t = p//2 at columns t*S2
            qk_chip(wk_op, 0, KDST, "sc", 0, 4, scalar_ok=True)
            qk_chip(wk_op, 0, KDST, "sc", 4, 8, scalar_ok=True)
            for c, tg in zip(range(4), ("pcA", "pcB", "sc", "pj")):
                qk_chip(wq_op, c, QDST, tg, 0, 4, scalar_ok=True)
            for c, tg in zip(range(4), ("pcA", "pcB", "sc", "pj")):
                qk_chip(wq_op, c, QDST, tg, 4, 8, scalar_ok=True)

            # ------------- static fill schedule (pass, mk) -> thunks -----
            # Every chip is <= 8 matmuls; placed so its inputs (DMA pieces,
            # the DVE reciprocal chain, normalized ctx) are ready before
            # the PE FIFO reaches it.
            SCHED = {}

            def at(pi, mk, fn):
                SCHED.setdefault((pi, mk), []).append(fn)

            # pass 0 (pr0-h0): k c1..3 and v0-lo just-in-time, then the
            # hi halves as their DMA pieces land.
            for i, c in enumerate((1, 2, 3)):
                at(0, i, lambda c=c: qk_chip(wk_op, c, KDST, "pj", 0, 8))
            at(0, 0, lambda: v_chip(0, 0, "pj", 0, 4))
            at(0, 1, lambda: v_chip(0, 0, "pj", 4, 8))
            for i, c in enumerate(range(4, 8)):
                at(0, 5 + i, lambda c=c: qk_chip(wk_op, c, KDST, "pj", 0, 8))
            at(0, 8, lambda: v_chip(0, 1, "pj", 0, 4))
            at(0, 9, lambda: v_chip(0, 1, "pj", 4, 8))
            for i, c in enumerate(range(4, 8)):
                at(0, 10 + i,
                   lambda c=c: qk_chip(wq_op, c, QDST, "pj", 0, 8))
            # v projections for later pairs, two chips each
            for p in range(1, NPAIR):
                pi0 = (p - 1) * 2 + 1          # previous pair's h1 pass
                at(pi0, 1, lambda p=p: v_chip(p, 0, "pj", 0, 4))
                at(pi0, 2, lambda p=p: v_chip(p, 0, "pj", 4, 8))
                at(pi0, 8, lambda p=p: v_chip(p, 1, "pj", 0, 4))
                at(pi0, 9, lambda p=p: v_chip(p, 1, "pj", 4, 8))
            # previous pair's normalize + out-projection (recip chain is
            # kicked off at this pass's mk0; ready by ~mk5)
            for pr in range(NPAIR - 1):
                pi = (pr + 1) * 2
                for qs in range(4):
                    at(pi, 8 + qs, lambda pr=pr, qs=qs: norm_qs(pr, qs))
                at(pi, 12, lambda pr=pr: op_chip(pr, 0, 0, 4))
                at(pi, 13, lambda pr=pr: op_chip(pr, 0, 4, 8))
                at(pi, 14, lambda pr=pr: op_chip(pr, 1, 0, 4))
                at(pi, 15, lambda pr=pr: op_chip(pr, 1, 4, 8))
            # last pair: quarters 0/1 normalize inside its final pass and
            # the out-projection's low c-chunks (ctx cols 0:512) follow
            at(7, 7, lambda: norm_qs(3, 0))
            at(7, 8, lambda: norm_qs(3, 1))
            at(7, 11, lambda: op_chip(3, 0, 0, 4))
            at(7, 13, lambda: op_chip(3, 1, 0, 4))

            # ------------- attention: 8 skewed passes --------------------
            def drain_pass(pc, pr, h2):
                """Drains of a finished pass: den rows (scalar), ctx rows
                (DVE, ahead of the reciprocal so the next pass's ctx isn't
                queued behind it), then the reciprocal chain."""
                if h2 == 0:
                    dens[pr] = cupool.tile([97, 512], F32, name="den",
                                           tag="den", bufs=2)
                for qh in range(2):
                    qs = h2 * 2 + qh
                    nc.vector.tensor_copy(dens[pr][qs * 32:qs * 32 + 1, :],
                                          pc[qh][64:65, :])
                    c_t = cupool.tile([64, 512], F32, name=f"cu{pr}{qs}",
                                      tag=f"cu{qs}", bufs=2)
                    nc.vector.tensor_copy(c_t[:], pc[qh][0:64, :])
                    cu[(pr, qs)] = c_t
                if pr == NPAIR - 1:
                    norm_recip_part(pr, h2)   # split halves for the tail
                elif h2 == 1:
                    norm_recip(pr)

            # two-iteration ctx skew: emit ctx(i-2) after scores(i), so
            # the in-order PE FIFO never parks on an exp semaphore and the
            # sem latency amortizes across the 2-slot sw ring.
            pend = []          # [(thunk, pass_info_if_last | None), ...]
            for pi in range(2 * NPAIR):
                pr, h2 = pi // 2, pi % 2
                t = pr // 2
                pc = [pspool.tile([65, 512], F32, name=f"pc{qh}",
                                  tag=("pcA", "pcB")[qh], bufs=1)
                      for qh in range(2)]
                for mk in range(NM):
                    sw = pspool.tile([128, 1024], F32, name="sw",
                                     tag="sc", bufs=2)
                    for qh in range(2):
                        nc.tensor.matmul(
                            sw[:, qh * 512:(qh + 1) * 512],
                            kTza[:, pr * S2 + mk * 128:
                                 pr * S2 + (mk + 1) * 128],
                            qT2a[:, (t * S2 + h2 * 1024 + qh * 512):
                                 (t * S2 + h2 * 1024 + (qh + 1) * 512)],
                            start=True, stop=True)
                    pT = ptpool.tile([128, 1024], F16, name="pT",
                                     tag="pt", bufs=6)
                    nc.scalar.activation(
                        pT[:], sw[:], mybir.ActivationFunctionType.Exp,
                        scale=float(SCALE))
                    if len(pend) == 2:
                        fn, fin = pend.pop(0)
                        fn()
                        if fin is not None:
                            drain_pass(*fin)

                    def make_ctx(pc=pc, pr=pr, mk=mk, pT=pT):
                        for qh in range(2):
                            nc.tensor.matmul(
                                pc[qh][:],
                                v65[pr][:, mk * 65:(mk + 1) * 65],
                                pT[:, qh * 512:(qh + 1) * 512],
                                start=(mk == 0), stop=(mk == NM - 1))
                    pend.append((make_ctx,
                                 (pc, pr, h2) if mk == NM - 1 else None))
                    for fn in SCHED.get((pi, mk), []):
                        fn()

            # flush remaining skewed ctx + drains (kicks recip part 1)
            for fn, fin in pend:
                fn()
                if fin is not None:
                    drain_pass(*fin)
            norm_qs(3, 2, tag="sc")
            norm_qs(3, 3, tag="sc")
            op_chip(3, 0, 4, 8)
            op_chip(3, 1, 4, 8)

    nc.compile()
    return nc


_CACHE = {}


def _get_nc():
    if "nc" not in _CACHE:
        _CACHE["nc"] = _build()
    return _CACHE["nc"]


def _kc_block(a, cols):
    """[1024, cols] -> [128, 8*cols] with kc blocks along columns."""
    return np.ascontiguousarray(
        a.reshape(KC, 128, cols).transpose(1, 0, 2).reshape(128, KC * cols))


def _half_block(a):
    """[1024, 1024] W.T -> [128, 8192] half-major: col h*4096 + kc*512 + f
    holds W.T[kc*128+p, h*512+f]."""
    return np.ascontiguousarray(
        a.reshape(KC, 128, 2, 512).transpose(1, 2, 0, 3).reshape(128, 8192))


def _prep_inputs(x, Wq, Wk, Wv, Wo):
    x = np.asarray(x, dtype=np.float32)
    wqa = _half_block(np.ascontiguousarray(Wq.T, dtype=np.float16))
    wka = _half_block(np.ascontiguousarray(Wk.T, dtype=np.float16))
    wva = _half_block(np.ascontiguousarray(Wv.T, dtype=np.float16))
    woa = _half_block(np.ascontiguousarray(Wo.T, dtype=np.float16))

    in_maps = []
    for core in range(NCORES):
        b, hg = core // 4, core % 4
        rows = x[b, hg * 512:(hg + 1) * 512, :]
        xTa = _kc_block(np.ascontiguousarray(rows.T.astype(np.float16)), 512)
        in_maps.append({
            "xTa": xTa, "wqa": wqa, "wka": wka, "wva": wva, "woa": woa,
        })
    return in_maps


def _run(in_maps, trace=False):
    nc = _get_nc()
    return run_bass_kernel_spmd(nc, in_maps, core_ids=list(range(NCORES)),
                                trace=trace)


def kernel(x, Wq, bq, Wk, bk, Wv, bv, Wo, bo, _trace=False):
    x = np.asarray(x, dtype=np.float32)
    in_maps = _prep_inputs(x, np.asarray(Wq), np.asarray(Wk),
                           np.asarray(Wv), np.asarray(Wo))
    res = _run(in_maps, trace=_trace)
    out = np.empty((B, S, D), dtype=np.float32)
    for core in range(NCORES):
        b, hg = core // 4, core % 4
        out[b, hg * 512:(hg + 1) * 512, :] = res.results[core]["out"]
    out += np.asarray(bo, dtype=np.float32)[None, None, :]
    kernel.last_result = res
    return out
